# revision 28
# baseline (speedup 1.0000x reference)
"""Graph-ODE (GCN message passing) Trainium2 kernel.

Problem: h0 = x @ W_fc + b_fc; 4 Euler steps of
  h <- h + 0.25 * relu(gcn2(relu(gcn1(h)))),  gcn(h) = (adj @ h) @ W + b
with B=32, N=4096, IN_DIM=64, H=128.

Strategy (8 NeuronCores, data-parallel over batch):
 - Each core owns 4 batches; adj (pre-transposed + tiled on host) and
   weights are replicated. No collectives.
 - Aggregation adj @ V runs with the ACTIVATION as the stationary
   operand (V slab [m,128h] per batch) and adjT as the moving operand
   ([m, 512n] chunks), in fp8-e4m3 DoubleRow (K=256/slab). The output
   lands PRE-TRANSPOSED [h, n] in PSUM, so the projection consumes it
   directly (stationary=aggT slab, moving=W) and emits z back in
   [n, h] node-major form. No PE transposes anywhere.
 - adj is scaled by 4096 on the host so entries sit in e4m3 normal
   range; the scale folds back via W/4096 in the projection.
 - SNAKE chunk ordering: 4 persistent chunk tiles; passes alternate
   direction (fwd/rev) so the 4 chunks resident at pass end are the
   first 4 the next pass needs — each pass after the first re-DMAs
   only 4 of 8 chunks (adjT traffic 134MB -> ~75MB/core).
 - Step-0 layer-1 aggregates x directly (adj@(x@Wfc) = (adj@x)@Wfc with
   W_fc@W1 folded on the host); two batches share one stationary.
 - Projections for unit k are emitted between the aggregation chains of
   unit k+1 so the PE never waits on the PSUM->SBUF drain.
 - Euler state h stays fp32 in SBUF; h0 = x@W_fc uses a packed hi/lo
   bf16 split: stationary [x_hi; x_lo] (K=128), two moving operands
   [Whi;Whi] and [Wlo;Wlo] -> exact 4-term product in 2 matmuls/tile
   (vs 3 matmuls at K=64 before).
 - Startup: first phase-0 unit's x rides split across the sync+scalar
   HW queues; chunk0/X8t stream in quarters interleaved with the early
   x units; dummy 0x0 warmup matmuls run during the initial DMA wait
   so the HAM clock gate is released (1.2->2.4GHz) before real work.
 - Remaining phase-0 units interleave into the x-layer with their x
   on the (otherwise idle) scalar queue, so they never stall behind
   2MB chunk transfers.
 - Outputs leave per-unit as contiguous-per-partition DMAs on the
   scalar HW queue; the final unit's drain is sliced per node-tile on
   sync so the kernel tail is short.
"""
import sys

sys.path.insert(0, "/opt/trn_rl_repo")

import numpy as np
import ml_dtypes

import concourse.bass as bass
import concourse.mybir as mybir
import concourse.tile as tile
from concourse.bass_utils import run_bass_kernel_spmd

BF16 = mybir.dt.bfloat16
FP8 = mybir.dt.float8e4
F32 = mybir.dt.float32
ADJ_SCALE = 4096.0

B, N, IN_DIM, H = 32, 4096, 64, 128
N_CORES = 8
BL = B // N_CORES          # 4 batches per core
NT = N // 128              # 32 node tiles
NCH = 8                    # 512-wide n chunks
CHW = N // NCH             # 512
STEP = 0.25
N_STEPS = 4
R_SLOTS = 5                # resident chunk slots (snake reuse)
WARM_MM = 12               # warmup matmuls during startup DMA wait


def _split_multiwait(nc):
    """This walrus build accepts only ONE sync-wait command per engine
    instruction (incl. drains). Hoist extra waits onto preceding
    single-wait InstNoOps on the same engine."""
    import bass_rust
    for fn in nc.m.functions:
        for blk in fn.blocks:
            out = []
            for inst in blk.instructions:
                si = inst.sync_info
                if (si is not None and si.on_wait and len(si.on_wait) > 1
                        and type(inst).__name__ not in (
                            "InstTensorLoad", "InstTensorSave", "InstTrigger")):
                    waits = list(si.on_wait)
                    for w in waits[:-1]:
                        out.append(mybir.InstNoOp(
                            name=nc.get_next_instruction_name(),
                            engine=inst.engine, ins=[], outs=[],
                            sync_info=bass_rust.SyncInfo(
                                on_wait=[w], on_update=[]),
                        ))
                    inst.sync_info = bass_rust.SyncInfo(
                        on_wait=[waits[-1]], on_update=list(si.on_update))
                out.append(inst)
            blk.instructions = out


def _build(with_bias):
    nc = bass.Bass()
    x_folded = not with_bias

    # adjT chunked: [chunk, p, mt2, i, n'] with m=(2*mt2+i)*128+p,
    # n = chunk*512+n'; scaled by ADJ_SCALE, fp8.
    adjm = nc.dram_tensor("adjm", [NCH, 128, NT // 2, 2, CHW], FP8,
                          kind="ExternalInput")
    if x_folded:
        x8 = nc.dram_tensor("x8", [128, NT, 2, 128], FP8, kind="ExternalInput")
        wfc1s = nc.dram_tensor("wfc1s", [128, H], BF16, kind="ExternalInput")
        xt_hi = nc.dram_tensor("xt_hi", [BL, IN_DIM, N], BF16,
                               kind="ExternalInput")
    else:
        xt_hi = nc.dram_tensor("xt_hi", [BL, IN_DIM, N], BF16,
                               kind="ExternalInput")
        xt_lo = nc.dram_tensor("xt_lo", [BL, IN_DIM, N], BF16,
                               kind="ExternalInput")
    wpack = nc.dram_tensor("wpack", [128, 512], BF16, kind="ExternalInput")
    if with_bias:
        b_fc = nc.dram_tensor("b_fc", [1, H], BF16, kind="ExternalInput")
        b1 = nc.dram_tensor("b1", [1, H], BF16, kind="ExternalInput")
        b2 = nc.dram_tensor("b2", [1, H], BF16, kind="ExternalInput")
        ones = nc.dram_tensor("ones", [1, H], BF16, kind="ExternalInput")
    out = nc.dram_tensor("out", [BL, 128, NT, H], F32, kind="ExternalOutput")

    relu = mybir.ActivationFunctionType.Relu
    DR = mybir.MatmulPerfMode.DoubleRow

    with tile.TileContext(nc) as tc:
        with tc.tile_pool(name="res", bufs=1) as res, \
             tc.tile_pool(name="wgt", bufs=1) as wgt, \
             tc.tile_pool(name="xs", bufs=6) as xs, \
             tc.tile_pool(name="work", bufs=3) as work, \
             tc.tile_pool(name="psA", bufs=4, space="PSUM") as psA, \
             tc.tile_pool(name="psZ", bufs=4, space="PSUM") as psZ:

            # --- resident state: fp8 activations in [p, mt, b, h]
            # node-major form (node m = mt*128 + p) to serve as matmul
            # stationaries; fp32 h in [p, b, mt, h] so each unit's final
            # output is one contiguous-per-partition DMA
            Hsb = res.tile([128, BL, NT, H], F32, tag="Hsb")
            Hbf = res.tile([128, NT, BL, H], FP8, tag="Hbf")
            Tbf = res.tile([128, NT, BL, H], FP8, tag="Tbf")
            # persistent chunk slots (snake reuse)
            chtiles = [res.tile([128, NT // 2, 2, CHW], FP8, tag=f"chs{i}",
                                name=f"chs{i}")
                       for i in range(R_SLOTS)]
            slot_of = {}

            # --- warmup: release the HAM clock gate during the initial
            # DMA wait with dummy matmuls on zeroed scratch. Must be
            # full-array (K=128) — thin matmuls don't register as
            # activity for the HAM window.
            warm = wgt.tile([128, 512], BF16, tag="warm")
            nc.gpsimd.memset(warm[:], 0)

            def emit_warmup(n):
                wpz = psZ.tile([128, 4, H], F32, tag="pz")
                for _ in range(n):
                    nc.tensor.matmul(wpz[:], warm[:, 0:128], warm[:],
                                     start=True, stop=True)

            emit_warmup(WARM_MM)

            # --- constants: fc part first on the HW queues (gates the
            # first phase-0 matmul), w1/w2 later on gpsimd.
            wpack_t = wgt.tile([128, 512], BF16, tag="wpack")
            w1_t = wpack_t[:, 0:128]
            w2_t = wpack_t[:, 128:256]
            if x_folded:
                nc.sync.dma_start(wpack_t[0:IN_DIM, 256:384],
                                  wpack[0:IN_DIM, 256:384])
                fcA = wpack_t[0:IN_DIM, 256:384]   # bf16(W_fc)
            else:
                nc.sync.dma_start(wpack_t[:], wpack[:])
                wfc_hi_t = wpack_t[0:IN_DIM, 256:384]
                wfc_lo_t = wpack_t[0:IN_DIM, 384:512]
            if with_bias:
                bfc_t = wgt.tile([1, H], BF16, tag="bfc")
                b1_t = wgt.tile([1, H], BF16, tag="b1")
                b2_t = wgt.tile([1, H], BF16, tag="b2")
                ones_t = wgt.tile([1, H], BF16, tag="ones")
                nc.sync.dma_start(bfc_t[:], b_fc[:])
                nc.sync.dma_start(b1_t[:], b1[:])
                nc.sync.dma_start(b2_t[:], b2[:])
                nc.sync.dma_start(ones_t[:], ones[:])

            # --- phase 0: h0 = x @ W_fc (+ b_fc).
            # x_folded: hi-only bf16, one K=64 matmul per 128-node tile.
            # (x's bf16 truncation costs ~2e-3 rel err on h0 — far
            # inside the tolerance; the fp8 aggregation path dominates
            # everything else anyway.)
            def emit_p0_unit(b, c, q=None):
                if x_folded:
                    xt = xs.tile([IN_DIM, CHW], BF16, tag="xh")
                    (q or nc.scalar).dma_start(
                        xt[:], xt_hi[b, :, bass.ts(c, CHW)])
                    pz = psZ.tile([128, 4, H], F32, tag="pz")
                    for j in range(4):
                        nc.tensor.matmul(pz[:, j, :], xt[:, bass.ts(j, 128)],
                                         fcA, start=True, stop=True)
                    nc.vector.tensor_copy(Hsb[:, b, bass.ts(c, 4), :], pz[:])
                else:
                    xh = xs.tile([IN_DIM, CHW], BF16, tag="xh")
                    xl = xs.tile([IN_DIM, CHW], BF16, tag="xl")
                    nc.sync.dma_start(xh[:], xt_hi[b, :, bass.ts(c, CHW)])
                    nc.scalar.dma_start(xl[:], xt_lo[b, :, bass.ts(c, CHW)])
                    pz = psZ.tile([128, 4, H], F32, tag="pz")
                    for j in range(4):
                        xhs = xh[:, bass.ts(j, 128)]
                        xls = xl[:, bass.ts(j, 128)]
                        nc.tensor.matmul(pz[:, j, :], xhs, wfc_hi_t,
                                         start=True, stop=False)
                        nc.tensor.matmul(pz[:, j, :], xls, wfc_hi_t,
                                         start=False, stop=False)
                        nc.tensor.matmul(pz[:, j, :], xhs, wfc_lo_t,
                                         start=False, stop=False)
                        nc.tensor.matmul(pz[:, j, :], ones_t[:], bfc_t[:],
                                         start=False, stop=True)
                    nc.vector.tensor_copy(Hsb[:, b, bass.ts(c, 4), :], pz[:])
                    nc.scalar.activation(
                        Hbf[:, bass.ts(c, 4), b, :], pz[:],
                        mybir.ActivationFunctionType.Copy)

            p0_iter = iter([(b, c) for b in range(BL) for c in range(NCH)])

            def emit_some_p0(n, **kw):
                for _ in range(n):
                    u = next(p0_iter, None)
                    if u is not None:
                        emit_p0_unit(*u, **kw)

            # --- chunk loads with snake-resident reuse
            def get_chunk(c, order, i, thirds=False):
                if c in slot_of:
                    return chtiles[slot_of[c]]
                if len(slot_of) < R_SLOTS:
                    s = len(slot_of)
                else:
                    s = slot_of.pop(order[i - R_SLOTS])
                slot_of[c] = s
                t = chtiles[s]
                if thirds:
                    # x-pass streams 14MB while the queues still ramp —
                    # spread each chunk over all three queues, ordered
                    # by the mt2 consumption sequence; chunk 1 is still
                    # inside the ramp, so stream it piecewise like ch0
                    if c == 1:
                        qs = [nc.sync, nc.scalar, nc.gpsimd]
                        for k, (lo, hi) in enumerate(
                                [(0, 2), (2, 4), (4, 6), (6, 9),
                                 (9, 12), (12, 16)]):
                            qs[k % 3].dma_start(t[:, lo:hi],
                                                adjm[c, :, lo:hi])
                    else:
                        nc.sync.dma_start(t[:, 0:6], adjm[c, :, 0:6])
                        nc.scalar.dma_start(t[:, 6:11], adjm[c, :, 6:11])
                        nc.gpsimd.dma_start(t[:, 11:16], adjm[c, :, 11:16])
                else:
                    half = NT // 4
                    nc.sync.dma_start(t[:, 0:half], adjm[c, :, 0:half])
                    nc.gpsimd.dma_start(t[:, half:], adjm[c, :, half:])
                return t

            # --- startup schedule: the x-layer pass runs FIRST — it
            # only needs chunk0 + X8t (~3MB); ALL phase-0 units
            # interleave into it (their Hsb slabs aren't consumed until
            # step0-layer2's projections, ~50us later). The PE bridges
            # the initial DMA window with warmup matmuls + 3 x units.
            if x_folded:
                slot_of[0] = 0
                ch0 = chtiles[0]
                X8t = res.tile([128, NT, 2, 128], FP8, tag="X8t")
                # The first chain is its own DMA bridge: ch0 streams as
                # 16 mt2-granular pieces round-robined across the three
                # queues in consumption order, so the chain starts as
                # soon as piece 0 lands and trickles forward per piece
                # (sub-us stalls — the HAM idle window never fires).
                emit_some_p0(1, q=nc.sync)
                emit_some_p0(1, q=nc.scalar)
                # gpsimd (slow-starting SW queue): weights first
                nc.gpsimd.dma_start(wpack_t[:, 0:256], wpack[:, 0:256])
                wfc1s_t = wgt.tile([128, H], BF16, tag="wfc1s")
                nc.gpsimd.dma_start(wfc1s_t[:], wfc1s[:])
                x8q = 0
                qs = [nc.sync, nc.scalar, nc.gpsimd]
                for piece in range(16):
                    if piece % 4 == 0:
                        # X8 quarter ahead of the pieces that need it
                        nc.scalar.dma_start(
                            X8t[:, bass.ts(x8q, 8), :, :],
                            x8[:, bass.ts(x8q, 8), :, :])
                        x8q += 1
                    qs[piece % 3].dma_start(ch0[:, piece:piece + 1],
                                            adjm[0, :, piece:piece + 1])
                # short bridge to the first piece
                emit_warmup(12)
            else:
                emit_some_p0(32)

            # Deferred-projection queue: the proj/drain of unit k is
            # emitted after the aggregation chains of unit k+1 so the
            # PE never waits on the PSUM->SBUF drain.
            pending = [None]

            def flush_pending():
                if pending[0] is not None:
                    pending[0]()
                    pending[0] = None

            pass_idx = 0

            # --- step0/layer1 via x: adj@(x@Wfc) = (adj@x)@Wfc with
            # W_fc@W1 folded on the host. Stationary = x slab
            # [m, 2 batches x 64 feats] so 64-wide features halve it.
            if x_folded:
                def xproj_f(c, bpair, pa):
                    def xproj():
                        ag = work.tile([128, CHW], BF16, tag="ag")
                        nc.vector.tensor_copy(ag[:], pa[:])
                        for bp in range(2):
                            b = 2 * bpair + bp
                            pz = psZ.tile([128, 4, H], F32, tag="pz")
                            for s in range(4):
                                nc.tensor.matmul(
                                    pz[:, s, :],
                                    ag[bass.ds(64 * bp, 64),
                                       bass.ts(s, 128)],
                                    wfc1s_t[bass.ds(64 * bp, 64), :],
                                    start=True, stop=True)
                            nc.scalar.activation(
                                Tbf[:, bass.ts(c, 4), b, :], pz[:], relu)
                    return xproj

                order = list(range(NCH))
                for i, c in enumerate(order):
                    ch = get_chunk(c, order, i, thirds=True)
                    if i < 2:
                        # ramp window: run both bpair chains interleaved
                        # piece-by-piece, so each arriving mt2 piece is
                        # consumed twice — halves the supply rate the
                        # still-ramping queues must sustain, and keeps
                        # the PE trickling (no HAM idle re-throttle).
                        pa0 = psA.tile([128, CHW], F32, tag="pa")
                        pa1 = psA.tile([128, CHW], F32, tag="pa")
                        for mt2 in range(NT // 2):
                            for bpair, pa in ((0, pa0), (1, pa1)):
                                nc.tensor.matmul(
                                    pa[:], X8t[:, bass.ts(mt2, 2), bpair, :],
                                    ch[:, mt2, :, :],
                                    start=(mt2 == 0),
                                    stop=(mt2 == NT // 2 - 1),
                                    perf_mode=DR)
                            if c == 0 and 2 <= mt2 <= 12:
                                emit_warmup(1)
                        for bpair, pa in ((0, pa0), (1, pa1)):
                            flush_pending()
                            pending[0] = xproj_f(c, bpair, pa)
                    else:
                        for bpair in range(2):
                            pa = psA.tile([128, CHW], F32, tag="pa")
                            for mt2 in range(NT // 2):
                                nc.tensor.matmul(
                                    pa[:], X8t[:, bass.ts(mt2, 2), bpair, :],
                                    ch[:, mt2, :, :],
                                    start=(mt2 == 0),
                                    stop=(mt2 == NT // 2 - 1),
                                    perf_mode=DR)
                            flush_pending()
                            pending[0] = xproj_f(c, bpair, pa)
                            emit_some_p0(2, q=nc.scalar)
                emit_some_p0(32, q=nc.scalar)
                pass_idx = 1

            # --- 4 Euler steps x 2 GCN layers, snake chunk order ---
            for step in range(N_STEPS):
                for layer in range(2):
                    if x_folded and step == 0 and layer == 0:
                        continue
                    V = Hbf if layer == 0 else Tbf
                    W = w1_t if layer == 0 else w2_t
                    bias = None if not with_bias else (b1_t if layer == 0 else b2_t)
                    last_pass = (step == N_STEPS - 1 and layer == 1)
                    order = (list(range(NCH)) if pass_idx % 2 == 0
                             else list(range(NCH))[::-1])
                    for i, c in enumerate(order):
                        ch = get_chunk(c, order, i)
                        for b in range(BL):
                            final_unit = (last_pass and i == NCH - 1
                                          and b == BL - 1)
                            pa = psA.tile([128, CHW], F32, tag="pa")
                            for mt2 in range(NT // 2):
                                nc.tensor.matmul(
                                    pa[:], V[:, bass.ts(mt2, 2), b, :],
                                    ch[:, mt2, :, :],
                                    start=(mt2 == 0), stop=(mt2 == NT // 2 - 1),
                                    perf_mode=DR)

                            def proj(c=c, b=b, pa=pa, W=W, bias=bias,
                                     layer=layer, step=step):
                                ag = work.tile([128, CHW], BF16, tag="ag")
                                nc.vector.tensor_copy(ag[:], pa[:])
                                pz = psZ.tile([128, 4, H], F32, tag="pz")
                                for s in range(4):
                                    nc.tensor.matmul(
                                        pz[:, s, :], ag[:, bass.ts(s, 128)], W,
                                        start=True, stop=bias is None)
                                    if bias is not None:
                                        nc.tensor.matmul(
                                            pz[:, s, :], ones_t[:], bias[:],
                                            start=False, stop=True)
                                if layer == 0:
                                    nc.scalar.activation(
                                        Tbf[:, bass.ts(c, 4), b, :], pz[:], relu)
                                else:
                                    tmp = work.tile([128, 4, H], F32, tag="tmp")
                                    nc.scalar.activation(tmp[:], pz[:], relu,
                                                         scale=STEP)
                                    nc.vector.tensor_add(
                                        Hsb[:, b, bass.ts(c, 4), :],
                                        Hsb[:, b, bass.ts(c, 4), :], tmp[:])
                                    if step == N_STEPS - 1:
                                        # final h: stream out as soon as
                                        # ready on the scalar HW queue.
                                        nc.scalar.dma_start(
                                            out[b, :, bass.ts(c, 4), :],
                                            Hsb[:, b, bass.ts(c, 4), :])
                                    else:
                                        nc.vector.tensor_copy(
                                            Hbf[:, bass.ts(c, 4), b, :],
                                            Hsb[:, b, bass.ts(c, 4), :])

                            def proj_final(c=c, b=b, pa=pa, W=W, bias=bias):
                                # last unit of the run: sliced per
                                # node-tile so ACT/add/DMA pipeline and
                                # the tail is short; pieces ride sync.
                                ag = work.tile([128, CHW], BF16, tag="ag")
                                pz = psZ.tile([128, 4, H], F32, tag="pz")
                                tmp = work.tile([128, 4, H], F32, tag="tmp")
                                for s in range(4):
                                    nc.vector.tensor_copy(
                                        ag[:, bass.ts(s, 128)],
                                        pa[:, bass.ts(s, 128)])
                                    nc.tensor.matmul(
                                        pz[:, s, :], ag[:, bass.ts(s, 128)], W,
                                        start=True, stop=bias is None)
                                    if bias is not None:
                                        nc.tensor.matmul(
                                            pz[:, s, :], ones_t[:], bias[:],
                                            start=False, stop=True)
                                    nc.scalar.activation(
                                        tmp[:, s, :], pz[:, s, :], relu,
                                        scale=STEP)
                                    nt_i = 4 * c + s
                                    nc.vector.tensor_add(
                                        Hsb[:, b, nt_i, :],
                                        Hsb[:, b, nt_i, :], tmp[:, s, :])
                                    nc.sync.dma_start(
                                        out[b, :, nt_i, :],
                                        Hsb[:, b, nt_i, :])

                            flush_pending()
                            if final_unit:
                                pending[0] = proj_final
                            else:
                                pending[0] = proj
                    pass_idx += 1
            flush_pending()

    _split_multiwait(nc)
    return nc


_NC_CACHE = {}


def _get_nc(with_bias):
    if with_bias not in _NC_CACHE:
        _NC_CACHE[with_bias] = _build(with_bias)
    return _NC_CACHE[with_bias]


def _bf(a):
    return np.ascontiguousarray(a.astype(ml_dtypes.bfloat16))


def _prep_in_maps(x, adj, W_fc, b_fc, W1, b1, W2, b2):
    x = np.asarray(x, dtype=np.float32)
    adj = np.asarray(adj, dtype=np.float32)
    W_fc = np.asarray(W_fc, dtype=np.float32)
    b_fc = np.asarray(b_fc, dtype=np.float32)
    W1 = np.asarray(W1, dtype=np.float32)
    b1 = np.asarray(b1, dtype=np.float32)
    W2 = np.asarray(W2, dtype=np.float32)
    b2 = np.asarray(b2, dtype=np.float32)

    with_bias = bool(np.any(b_fc) or np.any(b1) or np.any(b2))
    x_folded = not with_bias

    # adjT chunked for the moving operand: [chunk, p, mt, n'] with
    # m = mt*128 + p (mt dim viewed as [mt2, 2] pairs for DoubleRow).
    adjT = np.ascontiguousarray(adj.T) * ADJ_SCALE
    adjm = (adjT.reshape(NT, 128, N).transpose(1, 0, 2)      # [p, mt, n]
            .reshape(128, NT, NCH, CHW).transpose(2, 0, 1, 3))  # [c, p, mt, n']
    adjm = np.ascontiguousarray(adjm.reshape(NCH, 128, NT // 2, 2, CHW)
                                .astype(ml_dtypes.float8_e4m3))

    w1h, w2h = W1 / ADJ_SCALE, W2 / ADJ_SCALE
    wfc_hi = W_fc.astype(ml_dtypes.bfloat16).astype(np.float32)
    wfc_lo = W_fc - wfc_hi
    wpack = np.zeros((128, 512), dtype=np.float32)
    wpack[:, 0:128] = w1h
    wpack[:, 128:256] = w2h
    wpack[0:IN_DIM, 256:384] = wfc_hi
    wpack[0:IN_DIM, 384:512] = wfc_lo
    shared = {"adjm": adjm, "wpack": _bf(wpack)}
    if x_folded:
        wfc1 = (W_fc @ W1) / ADJ_SCALE
        wfc1s = np.zeros((128, H), dtype=np.float32)
        wfc1s[0:IN_DIM] = wfc1
        wfc1s[IN_DIM:128] = wfc1
        shared["wfc1s"] = _bf(wfc1s)
    if with_bias:
        shared.update({
            "b_fc": _bf(b_fc.reshape(1, H)),
            "b1": _bf(b1.reshape(1, H)),
            "b2": _bf(b2.reshape(1, H)),
            "ones": np.ones((1, H), dtype=ml_dtypes.bfloat16),
        })

    in_maps = []
    for cc in range(N_CORES):
        xs = x[cc * BL:(cc + 1) * BL]               # [BL, N, IN_DIM]
        xt = np.ascontiguousarray(xs.transpose(0, 2, 1))  # [BL, IN_DIM, N]
        xt_hi = xt.astype(ml_dtypes.bfloat16)
        if x_folded:
            m = {**shared, "xt_hi": np.ascontiguousarray(xt_hi)}
            # [p, mt, bpair, bp*64+f] with b = 2*bpair + bp, m = mt*128+p
            x8 = (xs.reshape(2, 2, NT, 128, IN_DIM)
                  .transpose(3, 2, 0, 1, 4).reshape(128, NT, 2, 128))
            m["x8"] = np.ascontiguousarray(x8.astype(ml_dtypes.float8_e4m3))
        else:
            xt_lo = _bf(xt - xt_hi.astype(np.float32))
            m = {**shared,
                 "xt_hi": np.ascontiguousarray(xt_hi),
                 "xt_lo": xt_lo}
        in_maps.append(m)
    return in_maps, with_bias


def gather(res):
    return np.concatenate(
        [np.asarray(res.results[c]["out"]).transpose(0, 2, 1, 3)
         .reshape(BL, N, H) for c in range(N_CORES)], axis=0)


def kernel(**inputs):
    in_maps, with_bias = _prep_in_maps(**inputs)
    nc = _get_nc(with_bias)
    res = run_bass_kernel_spmd(nc, in_maps, core_ids=list(range(N_CORES)))
    return gather(res)


def run_traced(**inputs):
    in_maps, with_bias = _prep_in_maps(**inputs)
    nc = _get_nc(with_bias)
    return run_bass_kernel_spmd(nc, in_maps, core_ids=list(range(N_CORES)),
                                trace=True)


# revision 31
# speedup vs baseline: 1.1922x; 1.1922x over previous
"""Graph-ODE (GCN message passing) Trainium2 kernel.

Problem: h0 = x @ W_fc + b_fc; 4 Euler steps of
  h <- h + 0.25 * relu(gcn2(relu(gcn1(h)))),  gcn(h) = (adj @ h) @ W + b
with B=32, N=4096, IN_DIM=64, H=128.

Strategy (8 NeuronCores, data-parallel over batch):
 - Each core owns 4 batches; adj (pre-transposed + tiled on host) and
   weights are replicated. No collectives.
 - Aggregation adj @ V runs with the ACTIVATION as the stationary
   operand (V slab [m,128h] per batch) and adjT as the moving operand
   ([m, 512n] chunks), in fp8-e4m3 DoubleRow (K=256/slab). The output
   lands PRE-TRANSPOSED [h, n] in PSUM, so the projection consumes it
   directly (stationary=aggT slab, moving=W) and emits z back in
   [n, h] node-major form. No PE transposes anywhere.
 - adj is scaled by 4096 on the host so entries sit in e4m3 normal
   range; the scale folds back via W/4096 in the projection.
 - SNAKE chunk ordering: 4 persistent chunk tiles; passes alternate
   direction (fwd/rev) so the 4 chunks resident at pass end are the
   first 4 the next pass needs — each pass after the first re-DMAs
   only 4 of 8 chunks (adjT traffic 134MB -> ~75MB/core).
 - Step-0 layer-1 aggregates x directly (adj@(x@Wfc) = (adj@x)@Wfc with
   W_fc@W1 folded on the host); two batches share one stationary.
 - Projections for unit k are emitted between the aggregation chains of
   unit k+1 so the PE never waits on the PSUM->SBUF drain.
 - Euler state h stays fp32 in SBUF; h0 = x@W_fc uses a packed hi/lo
   bf16 split: stationary [x_hi; x_lo] (K=128), two moving operands
   [Whi;Whi] and [Wlo;Wlo] -> exact 4-term product in 2 matmuls/tile
   (vs 3 matmuls at K=64 before).
 - Startup: first phase-0 unit's x rides split across the sync+scalar
   HW queues; chunk0/X8t stream in quarters interleaved with the early
   x units; dummy 0x0 warmup matmuls run during the initial DMA wait
   so the HAM clock gate is released (1.2->2.4GHz) before real work.
 - Remaining phase-0 units interleave into the x-layer with their x
   on the (otherwise idle) scalar queue, so they never stall behind
   2MB chunk transfers.
 - Outputs leave per-unit as contiguous-per-partition DMAs on the
   scalar HW queue; the final unit's drain is sliced per node-tile on
   sync so the kernel tail is short.
"""
import sys

sys.path.insert(0, "/opt/trn_rl_repo")

import numpy as np
import ml_dtypes

import concourse.bass as bass
import concourse.mybir as mybir
import concourse.tile as tile
from concourse.bass_utils import run_bass_kernel_spmd

BF16 = mybir.dt.bfloat16
FP8 = mybir.dt.float8e4
F32 = mybir.dt.float32
ADJ_SCALE = 4096.0

B, N, IN_DIM, H = 32, 4096, 64, 128
N_CORES = 8
BL = B // N_CORES          # 4 batches per core
NT = N // 128              # 32 node tiles
NCH = 8                    # 512-wide n chunks
CHW = N // NCH             # 512
STEP = 0.25
N_STEPS = 4
R_SLOTS = 5                # resident chunk slots (snake reuse)
WARM_MM = 12               # warmup matmuls during startup DMA wait


def _split_multiwait(nc):
    """This walrus build accepts only ONE sync-wait command per engine
    instruction (incl. drains). Hoist extra waits onto preceding
    single-wait InstNoOps on the same engine."""
    import bass_rust
    for fn in nc.m.functions:
        for blk in fn.blocks:
            out = []
            for inst in blk.instructions:
                si = inst.sync_info
                if (si is not None and si.on_wait and len(si.on_wait) > 1
                        and type(inst).__name__ not in (
                            "InstTensorLoad", "InstTensorSave", "InstTrigger")):
                    waits = list(si.on_wait)
                    for w in waits[:-1]:
                        out.append(mybir.InstNoOp(
                            name=nc.get_next_instruction_name(),
                            engine=inst.engine, ins=[], outs=[],
                            sync_info=bass_rust.SyncInfo(
                                on_wait=[w], on_update=[]),
                        ))
                    inst.sync_info = bass_rust.SyncInfo(
                        on_wait=[waits[-1]], on_update=list(si.on_update))
                out.append(inst)
            blk.instructions = out


def _build(with_bias):
    nc = bass.Bass()
    x_folded = not with_bias

    # adjT chunked: [chunk, p, mt2, i, n'] with m=(2*mt2+i)*128+p,
    # n = chunk*512+n'; scaled by ADJ_SCALE, fp8.
    adjm = nc.dram_tensor("adjm", [NCH, 128, NT // 2, 2, CHW], FP8,
                          kind="ExternalInput")
    if x_folded:
        x8 = nc.dram_tensor("x8", [128, NT, 2, 128], FP8, kind="ExternalInput")
        wfc1s = nc.dram_tensor("wfc1s", [128, H], BF16, kind="ExternalInput")
        xt_hi = nc.dram_tensor("xt_hi", [BL, IN_DIM, N], BF16,
                               kind="ExternalInput")
    else:
        xt_hi = nc.dram_tensor("xt_hi", [BL, IN_DIM, N], BF16,
                               kind="ExternalInput")
        xt_lo = nc.dram_tensor("xt_lo", [BL, IN_DIM, N], BF16,
                               kind="ExternalInput")
    wpack = nc.dram_tensor("wpack", [128, 512], BF16, kind="ExternalInput")
    if with_bias:
        b_fc = nc.dram_tensor("b_fc", [1, H], BF16, kind="ExternalInput")
        b1 = nc.dram_tensor("b1", [1, H], BF16, kind="ExternalInput")
        b2 = nc.dram_tensor("b2", [1, H], BF16, kind="ExternalInput")
        ones = nc.dram_tensor("ones", [1, H], BF16, kind="ExternalInput")
    out = nc.dram_tensor("out", [BL, 128, NT, H], F32, kind="ExternalOutput")

    relu = mybir.ActivationFunctionType.Relu
    DR = mybir.MatmulPerfMode.DoubleRow

    with tile.TileContext(nc) as tc:
        with tc.tile_pool(name="res", bufs=1) as res, \
             tc.tile_pool(name="wgt", bufs=1) as wgt, \
             tc.tile_pool(name="xs", bufs=6) as xs, \
             tc.tile_pool(name="work", bufs=3) as work, \
             tc.tile_pool(name="psA", bufs=4, space="PSUM") as psA, \
             tc.tile_pool(name="psZ", bufs=4, space="PSUM") as psZ:

            # --- resident state: fp8 activations in [p, mt, b, h]
            # node-major form (node m = mt*128 + p) to serve as matmul
            # stationaries; fp32 h in [p, b, mt, h] so each unit's final
            # output is one contiguous-per-partition DMA
            Hsb = res.tile([128, BL, NT, H], F32, tag="Hsb")
            Hbf = res.tile([128, NT, BL, H], FP8, tag="Hbf")
            Tbf = res.tile([128, NT, BL, H], FP8, tag="Tbf")
            # persistent chunk slots (snake reuse)
            chtiles = [res.tile([128, NT // 2, 2, CHW], FP8, tag=f"chs{i}",
                                name=f"chs{i}")
                       for i in range(R_SLOTS)]
            slot_of = {}

            # --- warmup: release the HAM clock gate during the initial
            # DMA wait with dummy matmuls on zeroed scratch. Must be
            # full-array (K=128) — thin matmuls don't register as
            # activity for the HAM window.
            warm = wgt.tile([128, 512], BF16, tag="warm")
            nc.gpsimd.memset(warm[:], 0)

            def emit_warmup(n):
                wpz = psZ.tile([128, 4, H], F32, tag="pz")
                for _ in range(n):
                    nc.tensor.matmul(wpz[:], warm[:, 0:128], warm[:],
                                     start=True, stop=True)

            emit_warmup(WARM_MM)

            # --- constants: fc part first on the HW queues (gates the
            # first phase-0 matmul), w1/w2 later on gpsimd.
            wpack_t = wgt.tile([128, 512], BF16, tag="wpack")
            w1_t = wpack_t[:, 0:128]
            w2_t = wpack_t[:, 128:256]
            if x_folded:
                nc.sync.dma_start(wpack_t[0:IN_DIM, 256:384],
                                  wpack[0:IN_DIM, 256:384])
                fcA = wpack_t[0:IN_DIM, 256:384]   # bf16(W_fc)
            else:
                nc.sync.dma_start(wpack_t[:], wpack[:])
                wfc_hi_t = wpack_t[0:IN_DIM, 256:384]
                wfc_lo_t = wpack_t[0:IN_DIM, 384:512]
            if with_bias:
                bfc_t = wgt.tile([1, H], BF16, tag="bfc")
                b1_t = wgt.tile([1, H], BF16, tag="b1")
                b2_t = wgt.tile([1, H], BF16, tag="b2")
                ones_t = wgt.tile([1, H], BF16, tag="ones")
                nc.sync.dma_start(bfc_t[:], b_fc[:])
                nc.sync.dma_start(b1_t[:], b1[:])
                nc.sync.dma_start(b2_t[:], b2[:])
                nc.sync.dma_start(ones_t[:], ones[:])

            # --- phase 0: h0 = x @ W_fc (+ b_fc).
            # x_folded: hi-only bf16, one K=64 matmul per 128-node tile.
            # (x's bf16 truncation costs ~2e-3 rel err on h0 — far
            # inside the tolerance; the fp8 aggregation path dominates
            # everything else anyway.)
            def emit_p0_unit(b, c, q=None):
                if x_folded:
                    xt = xs.tile([IN_DIM, CHW], BF16, tag="xh")
                    (q or nc.scalar).dma_start(
                        xt[:], xt_hi[b, :, bass.ts(c, CHW)])
                    pz = psZ.tile([128, 4, H], F32, tag="pz")
                    for j in range(4):
                        nc.tensor.matmul(pz[:, j, :], xt[:, bass.ts(j, 128)],
                                         fcA, start=True, stop=True)
                    nc.vector.tensor_copy(Hsb[:, b, bass.ts(c, 4), :], pz[:])
                else:
                    xh = xs.tile([IN_DIM, CHW], BF16, tag="xh")
                    xl = xs.tile([IN_DIM, CHW], BF16, tag="xl")
                    nc.sync.dma_start(xh[:], xt_hi[b, :, bass.ts(c, CHW)])
                    nc.scalar.dma_start(xl[:], xt_lo[b, :, bass.ts(c, CHW)])
                    pz = psZ.tile([128, 4, H], F32, tag="pz")
                    for j in range(4):
                        xhs = xh[:, bass.ts(j, 128)]
                        xls = xl[:, bass.ts(j, 128)]
                        nc.tensor.matmul(pz[:, j, :], xhs, wfc_hi_t,
                                         start=True, stop=False)
                        nc.tensor.matmul(pz[:, j, :], xls, wfc_hi_t,
                                         start=False, stop=False)
                        nc.tensor.matmul(pz[:, j, :], xhs, wfc_lo_t,
                                         start=False, stop=False)
                        nc.tensor.matmul(pz[:, j, :], ones_t[:], bfc_t[:],
                                         start=False, stop=True)
                    nc.vector.tensor_copy(Hsb[:, b, bass.ts(c, 4), :], pz[:])
                    nc.scalar.activation(
                        Hbf[:, bass.ts(c, 4), b, :], pz[:],
                        mybir.ActivationFunctionType.Copy)

            p0_iter = iter([(b, c) for b in range(BL) for c in range(NCH)])

            def emit_some_p0(n, **kw):
                for _ in range(n):
                    u = next(p0_iter, None)
                    if u is not None:
                        emit_p0_unit(*u, **kw)

            # --- chunk loads with snake-resident reuse
            def get_chunk(c, order, i, thirds=False):
                if c in slot_of:
                    return chtiles[slot_of[c]]
                if len(slot_of) < R_SLOTS:
                    s = len(slot_of)
                else:
                    s = slot_of.pop(order[i - R_SLOTS])
                slot_of[c] = s
                t = chtiles[s]
                if thirds:
                    # x-pass streams 14MB while the queues still ramp —
                    # spread each chunk over all three queues, ordered
                    # by the mt2 consumption sequence; chunk 1 is still
                    # inside the ramp, so stream it piecewise like ch0
                    if c == 1:
                        qs = [nc.sync, nc.scalar, nc.gpsimd]
                        for k, (lo, hi) in enumerate(
                                [(0, 2), (2, 4), (4, 6), (6, 9),
                                 (9, 12), (12, 16)]):
                            qs[k % 3].dma_start(t[:, lo:hi],
                                                adjm[c, :, lo:hi])
                    else:
                        nc.sync.dma_start(t[:, 0:6], adjm[c, :, 0:6])
                        nc.scalar.dma_start(t[:, 6:11], adjm[c, :, 6:11])
                        nc.gpsimd.dma_start(t[:, 11:16], adjm[c, :, 11:16])
                else:
                    half = NT // 4
                    nc.sync.dma_start(t[:, 0:half], adjm[c, :, 0:half])
                    nc.gpsimd.dma_start(t[:, half:], adjm[c, :, half:])
                return t

            # --- startup schedule: the x-layer pass runs FIRST — it
            # only needs chunk0 + X8t (~3MB); ALL phase-0 units
            # interleave into it (their Hsb slabs aren't consumed until
            # step0-layer2's projections, ~50us later). The PE bridges
            # the initial DMA window with warmup matmuls + 3 x units.
            if x_folded:
                slot_of[0] = 0
                ch0 = chtiles[0]
                X8t = res.tile([128, NT, 2, 128], FP8, tag="X8t")
                # The first chain is its own DMA bridge: ch0 streams as
                # 16 mt2-granular pieces round-robined across the three
                # queues in consumption order, so the chain starts as
                # soon as piece 0 lands and trickles forward per piece
                # (sub-us stalls — the HAM idle window never fires).
                emit_some_p0(1, q=nc.sync)
                emit_some_p0(1, q=nc.scalar)
                # gpsimd (slow-starting SW queue): weights first
                nc.gpsimd.dma_start(wpack_t[:, 0:256], wpack[:, 0:256])
                wfc1s_t = wgt.tile([128, H], BF16, tag="wfc1s")
                nc.gpsimd.dma_start(wfc1s_t[:], wfc1s[:])
                x8q = 0
                qs = [nc.sync, nc.scalar, nc.gpsimd]
                for piece in range(16):
                    if piece % 4 == 0:
                        # X8 quarter ahead of the pieces that need it
                        nc.scalar.dma_start(
                            X8t[:, bass.ts(x8q, 8), :, :],
                            x8[:, bass.ts(x8q, 8), :, :])
                        x8q += 1
                    qs[piece % 3].dma_start(ch0[:, piece:piece + 1],
                                            adjm[0, :, piece:piece + 1])
                # short bridge to the first piece
                emit_warmup(12)
            else:
                emit_some_p0(32)

            # Deferred-projection queue: the proj/drain of unit k is
            # emitted after the aggregation chains of unit k+1 so the
            # PE never waits on the PSUM->SBUF drain.
            pending = []

            def flush_pending():
                while pending:
                    pending.pop(0)()

            pass_idx = 0

            # --- step0/layer1 via x: adj@(x@Wfc) = (adj@x)@Wfc with
            # W_fc@W1 folded on the host. Stationary = x slab
            # [m, 2 batches x 64 feats] so 64-wide features halve it.
            if x_folded:
                def xproj_f(c, bpair, pa):
                    def xproj():
                        ag = work.tile([128, CHW], BF16, tag="ag")
                        nc.vector.tensor_copy(ag[:], pa[:])
                        for bp in range(2):
                            b = 2 * bpair + bp
                            pz = psZ.tile([128, 4, H], F32, tag="pz")
                            for s in range(4):
                                nc.tensor.matmul(
                                    pz[:, s, :],
                                    ag[bass.ds(64 * bp, 64),
                                       bass.ts(s, 128)],
                                    wfc1s_t[bass.ds(64 * bp, 64), :],
                                    start=True, stop=True)
                            nc.scalar.activation(
                                Tbf[:, bass.ts(c, 4), b, :], pz[:], relu)
                    return xproj

                order = list(range(NCH))
                for i, c in enumerate(order):
                    ch = get_chunk(c, order, i, thirds=True)
                    if i < 4:
                        # ramp window: run both bpair chains interleaved
                        # piece-by-piece, so each arriving mt2 piece is
                        # consumed twice — halves the supply rate the
                        # still-ramping queues must sustain, and keeps
                        # the PE trickling (no HAM idle re-throttle).
                        flush_pending()
                        pa0 = psA.tile([128, CHW], F32, tag="pa")
                        pa1 = psA.tile([128, CHW], F32, tag="pa")
                        for mt2 in range(NT // 2):
                            for bpair, pa in ((0, pa0), (1, pa1)):
                                nc.tensor.matmul(
                                    pa[:], X8t[:, bass.ts(mt2, 2), bpair, :],
                                    ch[:, mt2, :, :],
                                    start=(mt2 == 0),
                                    stop=(mt2 == NT // 2 - 1),
                                    perf_mode=DR)
                            if c == 0 and 2 <= mt2 <= 12:
                                emit_warmup(1)
                        pending.append(xproj_f(c, 0, pa0))
                        pending.append(xproj_f(c, 1, pa1))
                        if i >= 2:
                            emit_some_p0(2, q=nc.scalar)
                    else:
                        for bpair in range(2):
                            pa = psA.tile([128, CHW], F32, tag="pa")
                            for mt2 in range(NT // 2):
                                nc.tensor.matmul(
                                    pa[:], X8t[:, bass.ts(mt2, 2), bpair, :],
                                    ch[:, mt2, :, :],
                                    start=(mt2 == 0),
                                    stop=(mt2 == NT // 2 - 1),
                                    perf_mode=DR)
                            flush_pending()
                            pending.append(xproj_f(c, bpair, pa))
                            emit_some_p0(2, q=nc.scalar)
                emit_some_p0(32, q=nc.scalar)
                pass_idx = 1

            # --- 4 Euler steps x 2 GCN layers, snake chunk order ---
            for step in range(N_STEPS):
                for layer in range(2):
                    if x_folded and step == 0 and layer == 0:
                        continue
                    V = Hbf if layer == 0 else Tbf
                    W = w1_t if layer == 0 else w2_t
                    bias = None if not with_bias else (b1_t if layer == 0 else b2_t)
                    last_pass = (step == N_STEPS - 1 and layer == 1)
                    order = (list(range(NCH)) if pass_idx % 2 == 0
                             else list(range(NCH))[::-1])
                    for i, c in enumerate(order):
                        ch = get_chunk(c, order, i)
                        for b in range(BL):
                            final_unit = (last_pass and i == NCH - 1
                                          and b == BL - 1)
                            pa = psA.tile([128, CHW], F32, tag="pa")
                            for mt2 in range(NT // 2):
                                nc.tensor.matmul(
                                    pa[:], V[:, bass.ts(mt2, 2), b, :],
                                    ch[:, mt2, :, :],
                                    start=(mt2 == 0), stop=(mt2 == NT // 2 - 1),
                                    perf_mode=DR)

                            def proj(c=c, b=b, pa=pa, W=W, bias=bias,
                                     layer=layer, step=step):
                                ag = work.tile([128, CHW], BF16, tag="ag")
                                nc.vector.tensor_copy(ag[:], pa[:])
                                pz = psZ.tile([128, 4, H], F32, tag="pz")
                                for s in range(4):
                                    nc.tensor.matmul(
                                        pz[:, s, :], ag[:, bass.ts(s, 128)], W,
                                        start=True, stop=bias is None)
                                    if bias is not None:
                                        nc.tensor.matmul(
                                            pz[:, s, :], ones_t[:], bias[:],
                                            start=False, stop=True)
                                if layer == 0:
                                    nc.scalar.activation(
                                        Tbf[:, bass.ts(c, 4), b, :], pz[:], relu)
                                else:
                                    tmp = work.tile([128, 4, H], F32, tag="tmp")
                                    nc.scalar.activation(tmp[:], pz[:], relu,
                                                         scale=STEP)
                                    nc.vector.tensor_add(
                                        Hsb[:, b, bass.ts(c, 4), :],
                                        Hsb[:, b, bass.ts(c, 4), :], tmp[:])
                                    if step == N_STEPS - 1:
                                        # final h: stream out as soon as
                                        # ready on the scalar HW queue.
                                        nc.scalar.dma_start(
                                            out[b, :, bass.ts(c, 4), :],
                                            Hsb[:, b, bass.ts(c, 4), :])
                                    else:
                                        nc.vector.tensor_copy(
                                            Hbf[:, bass.ts(c, 4), b, :],
                                            Hsb[:, b, bass.ts(c, 4), :])

                            def proj_final(c=c, b=b, pa=pa, W=W, bias=bias):
                                # last unit of the run: sliced per
                                # node-tile so ACT/add/DMA pipeline and
                                # the tail is short; pieces ride sync.
                                ag = work.tile([128, CHW], BF16, tag="ag")
                                pz = psZ.tile([128, 4, H], F32, tag="pz")
                                tmp = work.tile([128, 4, H], F32, tag="tmp")
                                for s in range(4):
                                    nc.vector.tensor_copy(
                                        ag[:, bass.ts(s, 128)],
                                        pa[:, bass.ts(s, 128)])
                                    nc.tensor.matmul(
                                        pz[:, s, :], ag[:, bass.ts(s, 128)], W,
                                        start=True, stop=bias is None)
                                    if bias is not None:
                                        nc.tensor.matmul(
                                            pz[:, s, :], ones_t[:], bias[:],
                                            start=False, stop=True)
                                    nc.scalar.activation(
                                        tmp[:, s, :], pz[:, s, :], relu,
                                        scale=STEP)
                                    nt_i = 4 * c + s
                                    nc.vector.tensor_add(
                                        Hsb[:, b, nt_i, :],
                                        Hsb[:, b, nt_i, :], tmp[:, s, :])
                                    nc.sync.dma_start(
                                        out[b, :, nt_i, :],
                                        Hsb[:, b, nt_i, :])

                            flush_pending()
                            if final_unit:
                                pending.append(proj_final)
                            else:
                                pending.append(proj)
                    pass_idx += 1
            flush_pending()

    _split_multiwait(nc)
    return nc


_NC_CACHE = {}


def _get_nc(with_bias):
    if with_bias not in _NC_CACHE:
        _NC_CACHE[with_bias] = _build(with_bias)
    return _NC_CACHE[with_bias]


def _bf(a):
    return np.ascontiguousarray(a.astype(ml_dtypes.bfloat16))


def _prep_in_maps(x, adj, W_fc, b_fc, W1, b1, W2, b2):
    x = np.asarray(x, dtype=np.float32)
    adj = np.asarray(adj, dtype=np.float32)
    W_fc = np.asarray(W_fc, dtype=np.float32)
    b_fc = np.asarray(b_fc, dtype=np.float32)
    W1 = np.asarray(W1, dtype=np.float32)
    b1 = np.asarray(b1, dtype=np.float32)
    W2 = np.asarray(W2, dtype=np.float32)
    b2 = np.asarray(b2, dtype=np.float32)

    with_bias = bool(np.any(b_fc) or np.any(b1) or np.any(b2))
    x_folded = not with_bias

    # adjT chunked for the moving operand: [chunk, p, mt, n'] with
    # m = mt*128 + p (mt dim viewed as [mt2, 2] pairs for DoubleRow).
    adjT = np.ascontiguousarray(adj.T) * ADJ_SCALE
    adjm = (adjT.reshape(NT, 128, N).transpose(1, 0, 2)      # [p, mt, n]
            .reshape(128, NT, NCH, CHW).transpose(2, 0, 1, 3))  # [c, p, mt, n']
    adjm = np.ascontiguousarray(adjm.reshape(NCH, 128, NT // 2, 2, CHW)
                                .astype(ml_dtypes.float8_e4m3))

    w1h, w2h = W1 / ADJ_SCALE, W2 / ADJ_SCALE
    wfc_hi = W_fc.astype(ml_dtypes.bfloat16).astype(np.float32)
    wfc_lo = W_fc - wfc_hi
    wpack = np.zeros((128, 512), dtype=np.float32)
    wpack[:, 0:128] = w1h
    wpack[:, 128:256] = w2h
    wpack[0:IN_DIM, 256:384] = wfc_hi
    wpack[0:IN_DIM, 384:512] = wfc_lo
    shared = {"adjm": adjm, "wpack": _bf(wpack)}
    if x_folded:
        wfc1 = (W_fc @ W1) / ADJ_SCALE
        wfc1s = np.zeros((128, H), dtype=np.float32)
        wfc1s[0:IN_DIM] = wfc1
        wfc1s[IN_DIM:128] = wfc1
        shared["wfc1s"] = _bf(wfc1s)
    if with_bias:
        shared.update({
            "b_fc": _bf(b_fc.reshape(1, H)),
            "b1": _bf(b1.reshape(1, H)),
            "b2": _bf(b2.reshape(1, H)),
            "ones": np.ones((1, H), dtype=ml_dtypes.bfloat16),
        })

    in_maps = []
    for cc in range(N_CORES):
        xs = x[cc * BL:(cc + 1) * BL]               # [BL, N, IN_DIM]
        xt = np.ascontiguousarray(xs.transpose(0, 2, 1))  # [BL, IN_DIM, N]
        xt_hi = xt.astype(ml_dtypes.bfloat16)
        if x_folded:
            m = {**shared, "xt_hi": np.ascontiguousarray(xt_hi)}
            # [p, mt, bpair, bp*64+f] with b = 2*bpair + bp, m = mt*128+p
            x8 = (xs.reshape(2, 2, NT, 128, IN_DIM)
                  .transpose(3, 2, 0, 1, 4).reshape(128, NT, 2, 128))
            m["x8"] = np.ascontiguousarray(x8.astype(ml_dtypes.float8_e4m3))
        else:
            xt_lo = _bf(xt - xt_hi.astype(np.float32))
            m = {**shared,
                 "xt_hi": np.ascontiguousarray(xt_hi),
                 "xt_lo": xt_lo}
        in_maps.append(m)
    return in_maps, with_bias


def gather(res):
    return np.concatenate(
        [np.asarray(res.results[c]["out"]).transpose(0, 2, 1, 3)
         .reshape(BL, N, H) for c in range(N_CORES)], axis=0)


def kernel(**inputs):
    in_maps, with_bias = _prep_in_maps(**inputs)
    nc = _get_nc(with_bias)
    res = run_bass_kernel_spmd(nc, in_maps, core_ids=list(range(N_CORES)))
    return gather(res)


def run_traced(**inputs):
    in_maps, with_bias = _prep_in_maps(**inputs)
    nc = _get_nc(with_bias)
    return run_bass_kernel_spmd(nc, in_maps, core_ids=list(range(N_CORES)),
                                trace=True)


# revision 33
# speedup vs baseline: 1.1932x; 1.0009x over previous
"""Graph-ODE (GCN message passing) Trainium2 kernel.

Problem: h0 = x @ W_fc + b_fc; 4 Euler steps of
  h <- h + 0.25 * relu(gcn2(relu(gcn1(h)))),  gcn(h) = (adj @ h) @ W + b
with B=32, N=4096, IN_DIM=64, H=128.

Strategy (8 NeuronCores, data-parallel over batch):
 - Each core owns 4 batches; adj (pre-transposed + tiled on host) and
   weights are replicated. No collectives.
 - Aggregation adj @ V runs with the ACTIVATION as the stationary
   operand (V slab [m,128h] per batch) and adjT as the moving operand
   ([m, 512n] chunks), in fp8-e4m3 DoubleRow (K=256/slab). The output
   lands PRE-TRANSPOSED [h, n] in PSUM, so the projection consumes it
   directly (stationary=aggT slab, moving=W) and emits z back in
   [n, h] node-major form. No PE transposes anywhere.
 - adj is scaled by 4096 on the host so entries sit in e4m3 normal
   range; the scale folds back via W/4096 in the projection.
 - SNAKE chunk ordering: 4 persistent chunk tiles; passes alternate
   direction (fwd/rev) so the 4 chunks resident at pass end are the
   first 4 the next pass needs — each pass after the first re-DMAs
   only 4 of 8 chunks (adjT traffic 134MB -> ~75MB/core).
 - Step-0 layer-1 aggregates x directly (adj@(x@Wfc) = (adj@x)@Wfc with
   W_fc@W1 folded on the host); two batches share one stationary.
 - Projections for unit k are emitted between the aggregation chains of
   unit k+1 so the PE never waits on the PSUM->SBUF drain.
 - Euler state h stays fp32 in SBUF; h0 = x@W_fc uses a packed hi/lo
   bf16 split: stationary [x_hi; x_lo] (K=128), two moving operands
   [Whi;Whi] and [Wlo;Wlo] -> exact 4-term product in 2 matmuls/tile
   (vs 3 matmuls at K=64 before).
 - Startup: first phase-0 unit's x rides split across the sync+scalar
   HW queues; chunk0/X8t stream in quarters interleaved with the early
   x units; dummy 0x0 warmup matmuls run during the initial DMA wait
   so the HAM clock gate is released (1.2->2.4GHz) before real work.
 - Remaining phase-0 units interleave into the x-layer with their x
   on the (otherwise idle) scalar queue, so they never stall behind
   2MB chunk transfers.
 - Outputs leave per-unit as contiguous-per-partition DMAs on the
   scalar HW queue; the final unit's drain is sliced per node-tile on
   sync so the kernel tail is short.
"""
import sys

sys.path.insert(0, "/opt/trn_rl_repo")

import numpy as np
import ml_dtypes

import concourse.bass as bass
import concourse.mybir as mybir
import concourse.tile as tile
from concourse.bass_utils import run_bass_kernel_spmd

BF16 = mybir.dt.bfloat16
FP8 = mybir.dt.float8e4
F32 = mybir.dt.float32
ADJ_SCALE = 4096.0

B, N, IN_DIM, H = 32, 4096, 64, 128
N_CORES = 8
BL = B // N_CORES          # 4 batches per core
NT = N // 128              # 32 node tiles
NCH = 8                    # 512-wide n chunks
CHW = N // NCH             # 512
STEP = 0.25
N_STEPS = 4
R_SLOTS = 5                # resident chunk slots (snake reuse)
WARM_MM = 12               # warmup matmuls during startup DMA wait


def _split_multiwait(nc):
    """This walrus build accepts only ONE sync-wait command per engine
    instruction (incl. drains). Hoist extra waits onto preceding
    single-wait InstNoOps on the same engine."""
    import bass_rust
    for fn in nc.m.functions:
        for blk in fn.blocks:
            out = []
            for inst in blk.instructions:
                si = inst.sync_info
                if (si is not None and si.on_wait and len(si.on_wait) > 1
                        and type(inst).__name__ not in (
                            "InstTensorLoad", "InstTensorSave", "InstTrigger")):
                    waits = list(si.on_wait)
                    for w in waits[:-1]:
                        out.append(mybir.InstNoOp(
                            name=nc.get_next_instruction_name(),
                            engine=inst.engine, ins=[], outs=[],
                            sync_info=bass_rust.SyncInfo(
                                on_wait=[w], on_update=[]),
                        ))
                    inst.sync_info = bass_rust.SyncInfo(
                        on_wait=[waits[-1]], on_update=list(si.on_update))
                out.append(inst)
            blk.instructions = out


def _build(with_bias):
    nc = bass.Bass()
    x_folded = not with_bias

    # adjT chunked: [chunk, p, mt2, i, n'] with m=(2*mt2+i)*128+p,
    # n = chunk*512+n'; scaled by ADJ_SCALE, fp8.
    adjm = nc.dram_tensor("adjm", [NCH, 128, NT // 2, 2, CHW], FP8,
                          kind="ExternalInput")
    if x_folded:
        x8 = nc.dram_tensor("x8", [128, NT, 2, 128], FP8, kind="ExternalInput")
        wfc1s = nc.dram_tensor("wfc1s", [128, H], BF16, kind="ExternalInput")
        xt_hi = nc.dram_tensor("xt_hi", [BL, IN_DIM, N], BF16,
                               kind="ExternalInput")
    else:
        xt_hi = nc.dram_tensor("xt_hi", [BL, IN_DIM, N], BF16,
                               kind="ExternalInput")
        xt_lo = nc.dram_tensor("xt_lo", [BL, IN_DIM, N], BF16,
                               kind="ExternalInput")
    wpack = nc.dram_tensor("wpack", [128, 512], BF16, kind="ExternalInput")
    if with_bias:
        b_fc = nc.dram_tensor("b_fc", [1, H], BF16, kind="ExternalInput")
        b1 = nc.dram_tensor("b1", [1, H], BF16, kind="ExternalInput")
        b2 = nc.dram_tensor("b2", [1, H], BF16, kind="ExternalInput")
        ones = nc.dram_tensor("ones", [1, H], BF16, kind="ExternalInput")
    out = nc.dram_tensor("out", [BL, 128, NT, H], F32, kind="ExternalOutput")

    relu = mybir.ActivationFunctionType.Relu
    DR = mybir.MatmulPerfMode.DoubleRow

    with tile.TileContext(nc) as tc:
        with tc.tile_pool(name="res", bufs=1) as res, \
             tc.tile_pool(name="wgt", bufs=1) as wgt, \
             tc.tile_pool(name="xs", bufs=6) as xs, \
             tc.tile_pool(name="work", bufs=3) as work, \
             tc.tile_pool(name="psA", bufs=4, space="PSUM") as psA, \
             tc.tile_pool(name="psZ", bufs=4, space="PSUM") as psZ:

            # --- resident state: fp8 activations in [p, mt, b, h]
            # node-major form (node m = mt*128 + p) to serve as matmul
            # stationaries; fp32 h in [p, b, mt, h] so each unit's final
            # output is one contiguous-per-partition DMA
            Hsb = res.tile([128, BL, NT, H], F32, tag="Hsb")
            Hbf = res.tile([128, NT, BL, H], FP8, tag="Hbf")
            Tbf = res.tile([128, NT, BL, H], FP8, tag="Tbf")
            # persistent chunk slots (snake reuse)
            chtiles = [res.tile([128, NT // 2, 2, CHW], FP8, tag=f"chs{i}",
                                name=f"chs{i}")
                       for i in range(R_SLOTS)]
            slot_of = {}

            # --- warmup: release the HAM clock gate during the initial
            # DMA wait with dummy matmuls on zeroed scratch. Must be
            # full-array (K=128) — thin matmuls don't register as
            # activity for the HAM window.
            warm = wgt.tile([128, 512], BF16, tag="warm")
            nc.gpsimd.memset(warm[:], 0)

            def emit_warmup(n):
                wpz = psZ.tile([128, 4, H], F32, tag="pz")
                for _ in range(n):
                    nc.tensor.matmul(wpz[:], warm[:, 0:128], warm[:],
                                     start=True, stop=True)

            emit_warmup(WARM_MM)

            # --- constants: fc part first on the HW queues (gates the
            # first phase-0 matmul), w1/w2 later on gpsimd.
            wpack_t = wgt.tile([128, 512], BF16, tag="wpack")
            w1_t = wpack_t[:, 0:128]
            w2_t = wpack_t[:, 128:256]
            if x_folded:
                nc.sync.dma_start(wpack_t[0:IN_DIM, 256:384],
                                  wpack[0:IN_DIM, 256:384])
                fcA = wpack_t[0:IN_DIM, 256:384]   # bf16(W_fc)
            else:
                nc.sync.dma_start(wpack_t[:], wpack[:])
                wfc_hi_t = wpack_t[0:IN_DIM, 256:384]
                wfc_lo_t = wpack_t[0:IN_DIM, 384:512]
            if with_bias:
                bfc_t = wgt.tile([1, H], BF16, tag="bfc")
                b1_t = wgt.tile([1, H], BF16, tag="b1")
                b2_t = wgt.tile([1, H], BF16, tag="b2")
                ones_t = wgt.tile([1, H], BF16, tag="ones")
                nc.sync.dma_start(bfc_t[:], b_fc[:])
                nc.sync.dma_start(b1_t[:], b1[:])
                nc.sync.dma_start(b2_t[:], b2[:])
                nc.sync.dma_start(ones_t[:], ones[:])

            # --- phase 0: h0 = x @ W_fc (+ b_fc).
            # x_folded: hi-only bf16, one K=64 matmul per 128-node tile.
            # (x's bf16 truncation costs ~2e-3 rel err on h0 — far
            # inside the tolerance; the fp8 aggregation path dominates
            # everything else anyway.)
            def emit_p0_unit(b, c, q=None):
                if x_folded:
                    xt = xs.tile([IN_DIM, CHW], BF16, tag="xh")
                    (q or nc.scalar).dma_start(
                        xt[:], xt_hi[b, :, bass.ts(c, CHW)])
                    pz = psZ.tile([128, 4, H], F32, tag="pz")
                    for j in range(4):
                        nc.tensor.matmul(pz[:, j, :], xt[:, bass.ts(j, 128)],
                                         fcA, start=True, stop=True)
                    nc.vector.tensor_copy(Hsb[:, b, bass.ts(c, 4), :], pz[:])
                else:
                    xh = xs.tile([IN_DIM, CHW], BF16, tag="xh")
                    xl = xs.tile([IN_DIM, CHW], BF16, tag="xl")
                    nc.sync.dma_start(xh[:], xt_hi[b, :, bass.ts(c, CHW)])
                    nc.scalar.dma_start(xl[:], xt_lo[b, :, bass.ts(c, CHW)])
                    pz = psZ.tile([128, 4, H], F32, tag="pz")
                    for j in range(4):
                        xhs = xh[:, bass.ts(j, 128)]
                        xls = xl[:, bass.ts(j, 128)]
                        nc.tensor.matmul(pz[:, j, :], xhs, wfc_hi_t,
                                         start=True, stop=False)
                        nc.tensor.matmul(pz[:, j, :], xls, wfc_hi_t,
                                         start=False, stop=False)
                        nc.tensor.matmul(pz[:, j, :], xhs, wfc_lo_t,
                                         start=False, stop=False)
                        nc.tensor.matmul(pz[:, j, :], ones_t[:], bfc_t[:],
                                         start=False, stop=True)
                    nc.vector.tensor_copy(Hsb[:, b, bass.ts(c, 4), :], pz[:])
                    nc.scalar.activation(
                        Hbf[:, bass.ts(c, 4), b, :], pz[:],
                        mybir.ActivationFunctionType.Copy)

            p0_iter = iter([(b, c) for b in range(BL) for c in range(NCH)])

            def emit_some_p0(n, **kw):
                for _ in range(n):
                    u = next(p0_iter, None)
                    if u is not None:
                        emit_p0_unit(*u, **kw)

            # --- chunk loads with snake-resident reuse
            def get_chunk(c, order, i, thirds=False):
                if c in slot_of:
                    return chtiles[slot_of[c]]
                if len(slot_of) < R_SLOTS:
                    s = len(slot_of)
                else:
                    s = slot_of.pop(order[i - R_SLOTS])
                slot_of[c] = s
                t = chtiles[s]
                if thirds:
                    # x-pass streams 14MB while the queues still ramp —
                    # spread each chunk over all three queues, ordered
                    # by the mt2 consumption sequence; chunk 1 is still
                    # inside the ramp, so stream it piecewise like ch0
                    if c == 1:
                        qs = [nc.sync, nc.scalar, nc.gpsimd]
                        for k, (lo, hi) in enumerate(
                                [(0, 2), (2, 4), (4, 6), (6, 9),
                                 (9, 12), (12, 16)]):
                            qs[k % 3].dma_start(t[:, lo:hi],
                                                adjm[c, :, lo:hi])
                    else:
                        nc.sync.dma_start(t[:, 0:6], adjm[c, :, 0:6])
                        nc.scalar.dma_start(t[:, 6:11], adjm[c, :, 6:11])
                        nc.gpsimd.dma_start(t[:, 11:16], adjm[c, :, 11:16])
                else:
                    half = NT // 4
                    nc.sync.dma_start(t[:, 0:half], adjm[c, :, 0:half])
                    nc.gpsimd.dma_start(t[:, half:], adjm[c, :, half:])
                return t

            # --- startup schedule: the x-layer pass runs FIRST — it
            # only needs chunk0 + X8t (~3MB); ALL phase-0 units
            # interleave into it (their Hsb slabs aren't consumed until
            # step0-layer2's projections, ~50us later). The PE bridges
            # the initial DMA window with warmup matmuls + 3 x units.
            if x_folded:
                slot_of[0] = 0
                ch0 = chtiles[0]
                X8t = res.tile([128, NT, 2, 128], FP8, tag="X8t")
                # The first chain is its own DMA bridge: ch0 streams as
                # 16 mt2-granular pieces round-robined across the three
                # queues in consumption order, so the chain starts as
                # soon as piece 0 lands and trickles forward per piece
                # (sub-us stalls — the HAM idle window never fires).
                emit_some_p0(1, q=nc.sync)
                emit_some_p0(1, q=nc.scalar)
                # gpsimd (slow-starting SW queue): weights first
                nc.gpsimd.dma_start(wpack_t[:, 0:256], wpack[:, 0:256])
                wfc1s_t = wgt.tile([128, H], BF16, tag="wfc1s")
                nc.gpsimd.dma_start(wfc1s_t[:], wfc1s[:])
                x8q = 0
                qs = [nc.sync, nc.scalar, nc.gpsimd]
                for piece in range(16):
                    if piece % 4 == 0:
                        # X8 quarter ahead of the pieces that need it
                        nc.scalar.dma_start(
                            X8t[:, bass.ts(x8q, 8), :, :],
                            x8[:, bass.ts(x8q, 8), :, :])
                        x8q += 1
                    qs[piece % 3].dma_start(ch0[:, piece:piece + 1],
                                            adjm[0, :, piece:piece + 1])
                # short bridge to the first piece
                emit_warmup(12)
            else:
                emit_some_p0(32)

            # Deferred-projection queue: the proj/drain of unit k is
            # emitted after the aggregation chains of unit k+1 so the
            # PE never waits on the PSUM->SBUF drain.
            pending = []

            def flush_pending():
                while pending:
                    pending.pop(0)()

            pass_idx = 0

            # --- step0/layer1 via x: adj@(x@Wfc) = (adj@x)@Wfc with
            # W_fc@W1 folded on the host. Stationary = x slab
            # [m, 2 batches x 64 feats] so 64-wide features halve it.
            if x_folded:
                def xproj_f(c, bpair, pa):
                    def xproj():
                        ag = work.tile([128, CHW], BF16, tag="ag")
                        nc.vector.tensor_copy(ag[:], pa[:])
                        for bp in range(2):
                            b = 2 * bpair + bp
                            pz = psZ.tile([128, 4, H], F32, tag="pz")
                            for s in range(4):
                                nc.tensor.matmul(
                                    pz[:, s, :],
                                    ag[bass.ds(64 * bp, 64),
                                       bass.ts(s, 128)],
                                    wfc1s_t[bass.ds(64 * bp, 64), :],
                                    start=True, stop=True)
                            nc.scalar.activation(
                                Tbf[:, bass.ts(c, 4), b, :], pz[:], relu)
                    return xproj

                order = list(range(NCH))
                for i, c in enumerate(order):
                    ch = get_chunk(c, order, i, thirds=True)
                    if i < 4:
                        # ramp window: run both bpair chains interleaved
                        # piece-by-piece, so each arriving mt2 piece is
                        # consumed twice — halves the supply rate the
                        # still-ramping queues must sustain, and keeps
                        # the PE trickling (no HAM idle re-throttle).
                        flush_pending()
                        pa0 = psA.tile([128, CHW], F32, tag="pa")
                        pa1 = psA.tile([128, CHW], F32, tag="pa")
                        for mt2 in range(NT // 2):
                            for bpair, pa in ((0, pa0), (1, pa1)):
                                nc.tensor.matmul(
                                    pa[:], X8t[:, bass.ts(mt2, 2), bpair, :],
                                    ch[:, mt2, :, :],
                                    start=(mt2 == 0),
                                    stop=(mt2 == NT // 2 - 1),
                                    perf_mode=DR)
                            if c == 0 and mt2 >= 1:
                                emit_warmup(1)
                            elif c == 1 and mt2 % 2 == 0:
                                emit_warmup(1)
                        pending.append(xproj_f(c, 0, pa0))
                        pending.append(xproj_f(c, 1, pa1))
                        if i >= 2:
                            emit_some_p0(2, q=nc.scalar)
                    else:
                        for bpair in range(2):
                            pa = psA.tile([128, CHW], F32, tag="pa")
                            for mt2 in range(NT // 2):
                                nc.tensor.matmul(
                                    pa[:], X8t[:, bass.ts(mt2, 2), bpair, :],
                                    ch[:, mt2, :, :],
                                    start=(mt2 == 0),
                                    stop=(mt2 == NT // 2 - 1),
                                    perf_mode=DR)
                            flush_pending()
                            pending.append(xproj_f(c, bpair, pa))
                            emit_some_p0(2, q=nc.scalar)
                emit_some_p0(32, q=nc.scalar)
                pass_idx = 1

            # --- 4 Euler steps x 2 GCN layers, snake chunk order ---
            for step in range(N_STEPS):
                for layer in range(2):
                    if x_folded and step == 0 and layer == 0:
                        continue
                    V = Hbf if layer == 0 else Tbf
                    W = w1_t if layer == 0 else w2_t
                    bias = None if not with_bias else (b1_t if layer == 0 else b2_t)
                    last_pass = (step == N_STEPS - 1 and layer == 1)
                    order = (list(range(NCH)) if pass_idx % 2 == 0
                             else list(range(NCH))[::-1])
                    for i, c in enumerate(order):
                        ch = get_chunk(c, order, i)
                        for b in range(BL):
                            final_unit = (last_pass and i == NCH - 1
                                          and b >= BL - 2)
                            pa = psA.tile([128, CHW], F32, tag="pa")
                            for mt2 in range(NT // 2):
                                nc.tensor.matmul(
                                    pa[:], V[:, bass.ts(mt2, 2), b, :],
                                    ch[:, mt2, :, :],
                                    start=(mt2 == 0), stop=(mt2 == NT // 2 - 1),
                                    perf_mode=DR)

                            def proj(c=c, b=b, pa=pa, W=W, bias=bias,
                                     layer=layer, step=step):
                                ag = work.tile([128, CHW], BF16, tag="ag")
                                nc.vector.tensor_copy(ag[:], pa[:])
                                pz = psZ.tile([128, 4, H], F32, tag="pz")
                                for s in range(4):
                                    nc.tensor.matmul(
                                        pz[:, s, :], ag[:, bass.ts(s, 128)], W,
                                        start=True, stop=bias is None)
                                    if bias is not None:
                                        nc.tensor.matmul(
                                            pz[:, s, :], ones_t[:], bias[:],
                                            start=False, stop=True)
                                if layer == 0:
                                    nc.scalar.activation(
                                        Tbf[:, bass.ts(c, 4), b, :], pz[:], relu)
                                else:
                                    tmp = work.tile([128, 4, H], F32, tag="tmp")
                                    nc.scalar.activation(tmp[:], pz[:], relu,
                                                         scale=STEP)
                                    nc.vector.tensor_add(
                                        Hsb[:, b, bass.ts(c, 4), :],
                                        Hsb[:, b, bass.ts(c, 4), :], tmp[:])
                                    if step == N_STEPS - 1:
                                        # final h: stream out as soon as
                                        # ready on the scalar HW queue.
                                        nc.scalar.dma_start(
                                            out[b, :, bass.ts(c, 4), :],
                                            Hsb[:, b, bass.ts(c, 4), :])
                                    else:
                                        nc.vector.tensor_copy(
                                            Hbf[:, bass.ts(c, 4), b, :],
                                            Hsb[:, b, bass.ts(c, 4), :])

                            def proj_final(c=c, b=b, pa=pa, W=W, bias=bias):
                                # last unit of the run: sliced per
                                # node-tile so ACT/add/DMA pipeline and
                                # the tail is short; pieces ride sync.
                                ag = work.tile([128, CHW], BF16, tag="ag")
                                pz = psZ.tile([128, 4, H], F32, tag="pz")
                                tmp = work.tile([128, 4, H], F32, tag="tmp")
                                for s in range(4):
                                    nc.vector.tensor_copy(
                                        ag[:, bass.ts(s, 128)],
                                        pa[:, bass.ts(s, 128)])
                                    nc.tensor.matmul(
                                        pz[:, s, :], ag[:, bass.ts(s, 128)], W,
                                        start=True, stop=bias is None)
                                    if bias is not None:
                                        nc.tensor.matmul(
                                            pz[:, s, :], ones_t[:], bias[:],
                                            start=False, stop=True)
                                    nc.scalar.activation(
                                        tmp[:, s, :], pz[:, s, :], relu,
                                        scale=STEP)
                                    nt_i = 4 * c + s
                                    nc.vector.tensor_add(
                                        Hsb[:, b, nt_i, :],
                                        Hsb[:, b, nt_i, :], tmp[:, s, :])
                                    nc.sync.dma_start(
                                        out[b, :, nt_i, :],
                                        Hsb[:, b, nt_i, :])

                            flush_pending()
                            if final_unit:
                                pending.append(proj_final)
                            else:
                                pending.append(proj)
                    pass_idx += 1
            flush_pending()

    _split_multiwait(nc)
    return nc


_NC_CACHE = {}


def _get_nc(with_bias):
    if with_bias not in _NC_CACHE:
        _NC_CACHE[with_bias] = _build(with_bias)
    return _NC_CACHE[with_bias]


def _bf(a):
    return np.ascontiguousarray(a.astype(ml_dtypes.bfloat16))


def _prep_in_maps(x, adj, W_fc, b_fc, W1, b1, W2, b2):
    x = np.asarray(x, dtype=np.float32)
    adj = np.asarray(adj, dtype=np.float32)
    W_fc = np.asarray(W_fc, dtype=np.float32)
    b_fc = np.asarray(b_fc, dtype=np.float32)
    W1 = np.asarray(W1, dtype=np.float32)
    b1 = np.asarray(b1, dtype=np.float32)
    W2 = np.asarray(W2, dtype=np.float32)
    b2 = np.asarray(b2, dtype=np.float32)

    with_bias = bool(np.any(b_fc) or np.any(b1) or np.any(b2))
    x_folded = not with_bias

    # adjT chunked for the moving operand: [chunk, p, mt, n'] with
    # m = mt*128 + p (mt dim viewed as [mt2, 2] pairs for DoubleRow).
    adjT = np.ascontiguousarray(adj.T) * ADJ_SCALE
    adjm = (adjT.reshape(NT, 128, N).transpose(1, 0, 2)      # [p, mt, n]
            .reshape(128, NT, NCH, CHW).transpose(2, 0, 1, 3))  # [c, p, mt, n']
    adjm = np.ascontiguousarray(adjm.reshape(NCH, 128, NT // 2, 2, CHW)
                                .astype(ml_dtypes.float8_e4m3))

    w1h, w2h = W1 / ADJ_SCALE, W2 / ADJ_SCALE
    wfc_hi = W_fc.astype(ml_dtypes.bfloat16).astype(np.float32)
    wfc_lo = W_fc - wfc_hi
    wpack = np.zeros((128, 512), dtype=np.float32)
    wpack[:, 0:128] = w1h
    wpack[:, 128:256] = w2h
    wpack[0:IN_DIM, 256:384] = wfc_hi
    wpack[0:IN_DIM, 384:512] = wfc_lo
    shared = {"adjm": adjm, "wpack": _bf(wpack)}
    if x_folded:
        wfc1 = (W_fc @ W1) / ADJ_SCALE
        wfc1s = np.zeros((128, H), dtype=np.float32)
        wfc1s[0:IN_DIM] = wfc1
        wfc1s[IN_DIM:128] = wfc1
        shared["wfc1s"] = _bf(wfc1s)
    if with_bias:
        shared.update({
            "b_fc": _bf(b_fc.reshape(1, H)),
            "b1": _bf(b1.reshape(1, H)),
            "b2": _bf(b2.reshape(1, H)),
            "ones": np.ones((1, H), dtype=ml_dtypes.bfloat16),
        })

    in_maps = []
    for cc in range(N_CORES):
        xs = x[cc * BL:(cc + 1) * BL]               # [BL, N, IN_DIM]
        xt = np.ascontiguousarray(xs.transpose(0, 2, 1))  # [BL, IN_DIM, N]
        xt_hi = xt.astype(ml_dtypes.bfloat16)
        if x_folded:
            m = {**shared, "xt_hi": np.ascontiguousarray(xt_hi)}
            # [p, mt, bpair, bp*64+f] with b = 2*bpair + bp, m = mt*128+p
            x8 = (xs.reshape(2, 2, NT, 128, IN_DIM)
                  .transpose(3, 2, 0, 1, 4).reshape(128, NT, 2, 128))
            m["x8"] = np.ascontiguousarray(x8.astype(ml_dtypes.float8_e4m3))
        else:
            xt_lo = _bf(xt - xt_hi.astype(np.float32))
            m = {**shared,
                 "xt_hi": np.ascontiguousarray(xt_hi),
                 "xt_lo": xt_lo}
        in_maps.append(m)
    return in_maps, with_bias


def gather(res):
    return np.concatenate(
        [np.asarray(res.results[c]["out"]).transpose(0, 2, 1, 3)
         .reshape(BL, N, H) for c in range(N_CORES)], axis=0)


def kernel(**inputs):
    in_maps, with_bias = _prep_in_maps(**inputs)
    nc = _get_nc(with_bias)
    res = run_bass_kernel_spmd(nc, in_maps, core_ids=list(range(N_CORES)))
    return gather(res)


def run_traced(**inputs):
    in_maps, with_bias = _prep_in_maps(**inputs)
    nc = _get_nc(with_bias)
    return run_bass_kernel_spmd(nc, in_maps, core_ids=list(range(N_CORES)),
                                trace=True)
